# revision 27
# baseline (speedup 1.0000x reference)
"""Trainium2 Bass kernel for a GPT causal-attention block (v3).

Problem: y = proj(causal_attention(x @ W_attn)), B=4, T=2048, C=1024, 16 heads.
Sharding: 8 cores = 4 batches x 2 head-groups (8 heads each). Each core
computes its batch's attention for its 8 heads plus the partial projection
(W_proj rows of its heads); the host sums the two partials per batch.

v3 design (post-trace analysis of the v2 332us baseline):
  - x is transposed on the HOST: the kernel receives xT [C, T] and loses all
    128 PE transpose instructions (~35us PE) + their PSUM->SBUF casts.
  - S matmuls (K=64, one head pair) are emitted in 2-chunk batches of
    row-tiled (0,0)/(64,0) pairs with no full-array matmul in between: the
    PE packs adjacent same-mode tiled matmuls (HW-verified 108ns/MM vs 216
    serial; any interleaved full-array MM kills packing).
  - Diagonal-block restriction: for causal-diagonal chunks only the valid
    query range is computed in S (restricted N), exp (3D strided AP), and
    PV (restricted rhs stream); the mask multiply shrinks to the single
    128-wide diagonal block.
  - Softmax denominators: reciprocal_approx_fast straight off the PSUM den
    rows, then a 64-row-mode ones-row broadcast matmul pair (packs with the
    S batches) replaces v2's 64 serial K=1 broadcast matmuls.
"""
import sys, os, contextlib
from collections import deque

for _p in ("/opt/trn_rl_repo", "/root/.axon_site/_ro/trn_rl_repo"):
    if os.path.isdir(_p) and _p not in sys.path:
        sys.path.insert(0, _p)

import numpy as np

T, C, NHEAD, HS = 2048, 1024, 16, 64
NCORES = 8
HPC = NHEAD // 2          # heads per core = 8
DPC = HPC * HS            # head dims per core = 512
NCC = C // 128            # contraction chunks = 8
NQT = T // 512            # q tiles = 4
NPAIR = HPC // 2          # head pairs per core = 4
VLAG = 6                  # S->V chunk pipelining lag (chunks)

_CACHE = {}


def _build():
    import concourse.tile as tile
    import concourse.bass as bass
    from concourse import bacc, mybir

    f32 = mybir.dt.float32
    f32r = mybir.dt.float32r
    bf16 = mybir.dt.bfloat16
    FT = mybir.ActivationFunctionType
    from concourse.masks import make_identity

    nc = bacc.Bacc("TRN2", target_bir_lowering=False)
    xt_d = nc.declare_dram_parameter("xt", [C, T], bf16, isOutput=False)
    wq_d = nc.declare_dram_parameter("wq", [C, DPC], bf16, isOutput=False)
    wk_d = nc.declare_dram_parameter("wk", [C, DPC], bf16, isOutput=False)
    wv_d = nc.declare_dram_parameter("wv", [C, DPC], bf16, isOutput=False)
    wp_d = nc.declare_dram_parameter("wp", [DPC, C], bf16, isOutput=False)
    masks_d = nc.declare_dram_parameter("masks_c", [128, 256], bf16, isOutput=False)
    vones_d = nc.declare_dram_parameter("vones_c", [128, NPAIR, 33], bf16, isOutput=False)
    onesr_d = nc.declare_dram_parameter("onesr_c", [128, 128], bf16, isOutput=False)
    o_d = nc.declare_dram_parameter("o", [T, C], bf16, isOutput=True)

    xt_r = xt_d[:].rearrange("(n p) t -> p n t", p=128)   # [128, 8, 2048]
    o_r = o_d[:].rearrange("(n p) c -> n p c", p=128)

    with tile.TileContext(nc) as tc:
      with contextlib.ExitStack() as top:
        top.enter_context(nc.allow_low_precision(reason="bf16 pipeline, fp32 accum"))
        const = top.enter_context(tc.tile_pool(name="const", bufs=1))
        persist = top.enter_context(tc.tile_pool(name="persist", bufs=1))
        xpool = top.enter_context(tc.tile_pool(name="xpool", bufs=3))
        ppool = top.enter_context(tc.tile_pool(name="ppool", bufs=VLAG + 4))
        dpool = top.enter_context(tc.tile_pool(name="dpool", bufs=2))
        dbpool = top.enter_context(tc.tile_pool(name="dbpool", bufs=2))
        opool = top.enter_context(tc.tile_pool(name="opool", bufs=2))
        pss = top.enter_context(tc.tile_pool(name="pss", bufs=2, space="PSUM"))
        psy = top.enter_context(tc.tile_pool(name="psy", bufs=2, space="PSUM"))
        psmm = top.enter_context(tc.tile_pool(name="psmm", bufs=2, space="PSUM"))

        ident = const.tile([128, 128], bf16, tag="ident")
        make_identity(nc, ident)

        # HAM pre-warm during the initial DMA wait
        for _w in range(16):
            pwt = psmm.tile([128, 128], f32, tag="mm")
            nc.tensor.matmul(pwt[:], ident[:], ident[:], start=True, stop=True)

        # persistent tensors
        qT = [persist.tile([128, T], bf16, tag=f"qT{u}", name=f"qT{u}") for u in range(NPAIR)]
        kT = [persist.tile([128, T], bf16, tag=f"kT{u}", name=f"kT{u}") for u in range(NPAIR)]
        vp = [persist.tile([128, NPAIR, 161], bf16, tag=f"vp{t}", name=f"vp{t}")
              for t in range(T // 128)]
        yT = [persist.tile([128, T], bf16, tag=f"yT{u}", name=f"yT{u}") for u in range(NPAIR)]

        wq_sb = persist.tile([128, NCC, DPC], bf16, tag="wq")
        wk_sb = persist.tile([128, NCC, DPC], bf16, tag="wk")
        wv_sb = persist.tile([128, NCC, DPC], bf16, tag="wv")
        wp_sb = persist.tile([128, NPAIR, C], bf16, tag="wp")

        # ones rows for the 64-row-mode denominator broadcast matmuls
        onesr = const.tile([128, 128], bf16, tag="onesr")
        nc.sync.dma_start(out=onesr, in_=onesr_d[:])

        # ---------------- startup DMAs ----------------
        xq = {}   # quad -> [128, 8, 512] xT slice

        def dma_quad_x(q):
            xn = xpool.tile([128, NCC, 512], bf16, tag="xq")
            nc.sync.dma_start(out=xn, in_=xt_r[:, :, 512 * q:512 * (q + 1)])
            xq[q] = xn

        dma_quad_x(0)
        wqr = wq_d[:].rearrange("(n p) d -> p n d", p=128)
        wkr = wk_d[:].rearrange("(n p) d -> p n d", p=128)
        nc.sync.dma_start(out=wq_sb[:, :, 0:128], in_=wqr[:, :, 0:128])
        nc.sync.dma_start(out=wk_sb[:, :, 0:128], in_=wkr[:, :, 0:128])
        nc.sync.dma_start(out=wq_sb[:, :, 128:512], in_=wqr[:, :, 128:512])
        nc.sync.dma_start(out=wk_sb[:, :, 128:512], in_=wkr[:, :, 128:512])
        nc.sync.dma_start(out=wv_sb, in_=wv_d[:].rearrange("(n p) d -> p n d", p=128))
        masks = const.tile([128, 2, 128], bf16, tag="masks")
        nc.sync.dma_start(out=masks, in_=masks_d[:].rearrange("p (h n) -> p h n", h=2))
        for t in range(T // 128):
            nc.sync.dma_start(out=vp[t][:, :, 64:97], in_=vones_d[:])
        nc.sync.dma_start(out=wp_sb, in_=wp_d[:].rearrange("(n p) c -> p n c", p=128))

        # ---------------- qkv quad machinery ----------------
        def quad_closures(q):
            ops = []
            qs = slice(512 * q, 512 * (q + 1))

            def mk_qk(dt, which):
                w_sb, dst = (wq_sb, qT) if which == "q" else (wk_sb, kT)

                def f():
                    ps = psmm.tile([128, 512], f32, tag="mm")
                    for cc in range(NCC):
                        nc.tensor.matmul(ps[:], w_sb[:, cc, dt * 128:(dt + 1) * 128],
                                         xq[q][:, cc, :], start=(cc == 0), stop=(cc == NCC - 1))
                    nc.vector.tensor_copy(dst[dt][:, qs], ps[:])
                return f

            def mk_v(j):
                tt = 4 * q + j

                def f():
                    ps = psmm.tile([128, 512], f32, tag="mm")
                    for cc in range(NCC):
                        nc.tensor.matmul(ps[:], xq[q][:, cc, j * 128:(j + 1) * 128],
                                         wv_sb[:, cc, :], start=(cc == 0), stop=(cc == NCC - 1))
                    pv = ps.rearrange("p (u two d) -> p u two d", u=NPAIR, two=2)
                    nc.vector.tensor_copy(vp[tt][:, :, 0:64], pv[:, :, 0, :])
                    nc.vector.tensor_copy(vp[tt][:, :, 97:161], pv[:, :, 1, :])
                return f

            for dt in range(4):
                ops.append((("qk", q, dt, "q"), mk_qk(dt, "q")))
                ops.append((("qk", q, dt, "k"), mk_qk(dt, "k")))
            for j in range(4):
                ops.append((("v", q, j), mk_v(j)))
            return ops

        # ---------------- attention ----------------
        pending_norm = deque()   # deferred 64-row-mode norm MM pairs + muls

        def emit_pending_norms():
            while pending_norm:
                fn = pending_norm.popleft()
                fn()

        def mk_norm(u, i, dinvb):
            qs = slice(512 * i, 512 * (i + 1))

            def f():
                # packed 64-row-mode pair: replicate 1/den_e (row 64) and
                # 1/den_o (row 32) across all psum partitions
                rbE = psmm.tile([128, 512], f32, tag="mm")
                nc.tensor.matmul(rbE[:], onesr[64:128, :], dinvb[64:128, :],
                                 start=True, stop=True, tile_position=(64, 0))
                rbO = psmm.tile([128, 512], f32, tag="mm")
                nc.tensor.matmul(rbO[:], onesr[0:64, :], dinvb[0:64, :],
                                 start=True, stop=True, tile_position=(0, 0))
                nc.vector.tensor_mul(yT[u][0:64, qs], yT[u][0:64, qs], rbE[0:64, :])
                nc.vector.tensor_mul(yT[u][64:128, qs], yT[u][64:128, qs], rbO[64:128, :])
            return f

        def emit_attn(u, i, pop):
            L = 4 * (i + 1)
            qs = slice(512 * i, 512 * (i + 1))
            ps_e = psy.tile([128, 512], f32, tag="ps_y")
            ps_o = psy.tile([128, 512], f32, tag="ps_y")
            Pt = {}
            Nc = {}   # chunk -> valid q-column start within the 512-wide tile

            def pv(c):
                lo = Nc[c]
                nc.tensor.matmul(ps_e[0:65, lo:512], vp[c][:, u, 0:65],
                                 Pt[c][:, lo:512],
                                 start=(c == 0), stop=(c == L - 1))
                nc.tensor.matmul(ps_o[:, lo:512], vp[c][:, u, 33:161],
                                 Pt[c][:, 512 + lo:1024],
                                 start=(c == 0), stop=(c == L - 1))

            for g in range(L // 2):
                c0, c1 = 2 * g, 2 * g + 1
                # ---- 64-row-mode batch: pending norms + two S pairs ----
                # (flush at g==1 so the previous unit's reciprocal has had a
                # full group of slack before the PE hits the rb matmuls)
                if g == 1:
                    emit_pending_norms()
                sps = {}
                for c in (c0, c1):
                    j = c - 4 * i
                    Nc[c] = 128 * j if j > 0 else 0
                    # tiny N=1 64-row-mode first writer: absorbs the PSUM
                    # slot-reuse wait so the real S quad below runs wait-free
                    # (a mid-quad wait serializes the row-tiled packing)
                    sp = pss.tile([128, 1024], f32, tag="s_pair")
                    nc.tensor.matmul(sp[:, 512:513], kT[u][64:128, 0:128],
                                     qT[u][64:128, 512 * i:512 * i + 1],
                                     start=True, stop=True, tile_position=(64, 0))
                    sps[c] = sp
                for c in (c0, c1):
                    lo = Nc[c]
                    sp = sps[c]
                    nc.tensor.matmul(sp[:, lo:512], kT[u][0:64, c * 128:(c + 1) * 128],
                                     qT[u][0:64, 512 * i + lo:512 * (i + 1)],
                                     start=True, stop=True, tile_position=(0, 0))
                    nc.tensor.matmul(sp[:, 512 + lo:1024], kT[u][64:128, c * 128:(c + 1) * 128],
                                     qT[u][64:128, 512 * i + lo:512 * (i + 1)],
                                     start=True, stop=True, tile_position=(64, 0))
                # ---- exp + masks ----
                for c in (c0, c1):
                    j = c - 4 * i
                    lo = Nc[c]
                    P = ppool.tile([128, 1024], bf16, tag="P")
                    if lo == 0:
                        nc.scalar.activation(out=P[:], in_=sps[c][:], func=FT.Exp,
                                             scale=float(HS) ** -0.5)
                    else:
                        nc.scalar.activation(out=P[:, lo:512], in_=sps[c][:, lo:512],
                                             func=FT.Exp, scale=float(HS) ** -0.5)
                        nc.scalar.activation(out=P[:, 512 + lo:1024],
                                             in_=sps[c][:, 512 + lo:1024],
                                             func=FT.Exp, scale=float(HS) ** -0.5)
                    if j >= 0:
                        # causal mask on the 128-wide diagonal block via
                        # gpsimd (idle engine; keeps PV's dep off the DVE
                        # queue): keep col >= row, else 0
                        p3 = P.rearrange("p (h n) -> p h n", h=2)
                        nc.gpsimd.affine_select(
                            out=p3[:, :, lo:lo + 128], in_=p3[:, :, lo:lo + 128],
                            pattern=[[0, 2], [1, 128]],
                            compare_op=mybir.AluOpType.is_ge, fill=0.0,
                            base=0, channel_multiplier=-1)
                    Pt[c] = P
                # ---- lagged PV (full-array mode) ----
                if c1 - VLAG >= 0:
                    pv(c0 - VLAG)
                    pv(c1 - VLAG)
                pop()
            for c in range(max(0, L - VLAG), L):
                pv(c)
            # ---- epilogue: recips via staged SBUF den rows, unnormalized yT ----
            # (single full-tile recip at base partition 0: custom DVE ops at
            # a nonzero base partition mis-lower and corrupt SBUF)
            dtmp = dpool.tile([128, 512], f32, tag="dtmp")
            drec = dpool.tile([128, 512], f32, tag="drec")
            nc.gpsimd.memset(dtmp[:], 1.0)
            nc.vector.tensor_copy(dtmp[64:65, :], ps_e[64:65, :])
            nc.vector.tensor_copy(dtmp[32:33, :], ps_o[32:33, :])
            nc.vector.reciprocal_approx_fast(out=drec[:], in_=dtmp[:])
            dinvb = dbpool.tile([128, 512], bf16, tag="dinvb")
            nc.gpsimd.memset(dinvb[:], 1.0)
            nc.vector.tensor_copy(dinvb[32:33, :], drec[32:33, :])
            nc.vector.tensor_copy(dinvb[64:65, :], drec[64:65, :])
            nc.vector.tensor_copy(yT[u][0:64, qs], ps_e[0:64, :])
            nc.vector.tensor_copy(yT[u][64:128, qs], ps_o[64:128, :])
            pending_norm.append(mk_norm(u, i, dinvb))

        # ---------------- projection ----------------
        def proj_closures(i):
            ops = []

            def mk_proj(tt):
                def f():
                    out_sb = opool.tile([128, C], bf16, tag="out_sb")
                    for ct in range(2):
                        po = psmm.tile([128, 512], f32, tag="mm")
                        for u in range(NPAIR):
                            nc.tensor.matmul(po[:], yT[u][:, tt * 128:(tt + 1) * 128],
                                             wp_sb[:, u, ct * 512:(ct + 1) * 512],
                                             start=(u == 0), stop=(u == NPAIR - 1))
                        nc.vector.tensor_copy(out_sb[:, ct * 512:(ct + 1) * 512], po[:])
                    nc.sync.dma_start(out=o_r[tt], in_=out_sb)
                return f

            for tt in range(4 * i, 4 * i + 4):
                ops.append((("proj", i, tt), mk_proj(tt)))
            return ops

        # ---------------- emission schedule ----------------
        q0 = quad_closures(0)
        prologue_keys = {("qk", 0, 0, "q"), ("qk", 0, 0, "k")}
        prologue_keys |= {("v", 0, j) for j in range(4)}
        rest0 = []
        for key, cl in q0:
            if key in prologue_keys:
                cl()
            else:
                rest0.append((key, cl))

        fillers = deque(rest0)

        def make_pop(total_groups, spill=0):
            state = {"left": total_groups + spill}

            def pop():
                n = state["left"]
                if n > 0:
                    k = len(fillers) // n
                    for _ in range(min(k, len(fillers))):
                        key, cl = fillers.popleft()
                        cl()
                    state["left"] = n - 1
            return pop

        dma_quad_x(1)
        for i in range(NQT):
            if i < NQT - 2:
                dma_quad_x(i + 2)
            if i < NQT - 1:
                fillers.extend(quad_closures(i + 1))
            if i > 0:
                # stage i-1's yT fully normalized once its last unit's norm
                # flushes; then its proj closures can pop as fillers
                emit_pending_norms()
                fillers.extend(proj_closures(i - 1))
            pop = make_pop(NPAIR * 2 * (i + 1), spill=6)
            for u in range(NPAIR):
                need = {("qk", qq, u, w) for qq in range(i + 1) for w in ("q", "k")}
                need |= {("v", qq, j) for qq in range(i + 1) for j in range(4)}
                while any(key in need for key, _ in fillers):
                    key, cl = fillers.popleft()
                    cl()
                emit_attn(u, i, pop)
        # tail: drain leftovers first (covers the last norm's recip latency),
        # then the pending norm, then the final projection
        while fillers:
            key, cl = fillers.popleft()
            cl()
        emit_pending_norms()
        for key, cl in proj_closures(NQT - 1):
            cl()

    nc.compile()
    return nc


def _get_nc():
    if "nc" not in _CACHE:
        _CACHE["nc"] = _build()
    return _CACHE["nc"]


def _in_maps(x, W_attn, W_proj):
    import ml_dtypes
    bf16 = ml_dtypes.bfloat16
    a_idx = np.arange(128)[:, None]
    b_idx = np.arange(128)[None, :]
    m = (b_idx >= a_idx).astype(np.float32)                 # [128k, 128q] valid q>=k
    masks_c = np.ascontiguousarray(
        np.concatenate([m, m], axis=1).astype(bf16))        # [128, 256]
    vones_c = np.ones((128, NPAIR, 33), bf16)
    onesr_c = np.zeros((128, 128), np.float32)
    onesr_c[32, :] = 1.0
    onesr_c[64, :] = 1.0
    onesr_c = np.ascontiguousarray(onesr_c.astype(bf16))
    maps = []
    for core in range(NCORES):
        b, g = core // 2, core % 2
        cs = slice(DPC * g, DPC * (g + 1))
        maps.append({
            "xt": np.ascontiguousarray(x[b].T.astype(bf16)),
            "wq": np.ascontiguousarray(W_attn[:, cs].astype(bf16)),
            "wk": np.ascontiguousarray(W_attn[:, C:][:, cs].astype(bf16)),
            "wv": np.ascontiguousarray(W_attn[:, 2 * C:][:, cs].astype(bf16)),
            "wp": np.ascontiguousarray(W_proj[cs, :].astype(bf16)),
            "masks_c": masks_c,
            "vones_c": vones_c,
            "onesr_c": onesr_c,
        })
    return maps


def _install_ntff_shim():
    """Provide antenv.axon_hooks (absent in this image) so trace=True works."""
    import sys as _sys, types, ctypes, contextlib as _cl
    if "antenv.axon_hooks" in _sys.modules:
        return
    so_path = "/opt/axon/libaxon_pjrt.so"
    try:
        lib = ctypes.CDLL(so_path)
        lib.axon_start_nrt_profile.argtypes = [ctypes.POINTER(ctypes.c_int64), ctypes.c_size_t]
        lib.axon_start_nrt_profile.restype = ctypes.c_int64
        lib.axon_stop_nrt_profile.argtypes = [ctypes.c_char_p]
        lib.axon_stop_nrt_profile.restype = ctypes.c_int64
    except (OSError, AttributeError):
        return

    @_cl.contextmanager
    def _hook(output_dir, device_ids):
        import jax
        jax.devices()
        if device_ids:
            ids = (ctypes.c_int64 * len(device_ids))(*device_ids)
            rc = lib.axon_start_nrt_profile(ids, len(device_ids))
        else:
            rc = lib.axon_start_nrt_profile(None, 0)
        if rc != 0:
            raise RuntimeError(f"axon_start_nrt_profile rc={rc}")
        try:
            yield
        finally:
            n = lib.axon_stop_nrt_profile(str(output_dir).encode())
            if n < 0:
                raise RuntimeError(f"axon_stop_nrt_profile rc={n}")

    mod = types.ModuleType("antenv.axon_hooks")
    mod.get_axon_ntff_profile_hook = lambda: _hook
    mod.set_axon_ntff_profile_hook = lambda h: None
    _sys.modules["antenv.axon_hooks"] = mod


def kernel(x, W_attn, W_proj, _trace=False):
    from concourse.bass_utils import run_bass_kernel_spmd
    if _trace:
        _install_ntff_shim()
    x = np.asarray(x, dtype=np.float32)
    W_attn = np.asarray(W_attn, dtype=np.float32)
    W_proj = np.asarray(W_proj, dtype=np.float32)
    nc = _get_nc()
    res = run_bass_kernel_spmd(nc, _in_maps(x, W_attn, W_proj),
                               core_ids=list(range(NCORES)), trace=_trace)
    out = np.empty((4, T, C), np.float32)
    for b in range(4):
        out[b] = (res.results[2 * b]["o"].astype(np.float32)
                  + res.results[2 * b + 1]["o"].astype(np.float32))
    if _trace:
        return out, res
    return out


# revision 29
# speedup vs baseline: 1.0113x; 1.0113x over previous
"""Trainium2 Bass kernel for a GPT causal-attention block (v3).

Problem: y = proj(causal_attention(x @ W_attn)), B=4, T=2048, C=1024, 16 heads.
Sharding: 8 cores = 4 batches x 2 head-groups (8 heads each). Each core
computes its batch's attention for its 8 heads plus the partial projection
(W_proj rows of its heads); the host sums the two partials per batch.

v3 design (post-trace analysis of the v2 332us baseline; 332 -> 310us):
  - x is transposed on the HOST: the kernel receives xT [C, T] and loses all
    128 PE transpose instructions (~35us PE) + their PSUM->SBUF casts.
  - Diagonal restriction: for causal-diagonal chunks only the valid query
    range is computed in S (restricted N), exp (sliced per head), and PV
    (restricted rhs stream); ~20us less ACT-exp. The causal mask shrinks to
    the single 128-wide diagonal block and runs as a gpsimd affine_select
    (idle engine; keeps PV's dependency off the DVE queue).
  - Softmax denominators: one reciprocal_approx_fast per unit (full-tile at
    base partition 0 -- custom DVE ops at a nonzero base partition mis-lower
    and corrupt SBUF), then a 64-row-mode ones-row broadcast matmul pair
    replaces v2's 64 serial K=1 broadcast matmuls + 3.3us reciprocals.
  - proj(i) closures released as fillers at stage i+1 (v2 held them to the
    last stage; the tail ran ~45us at 55% PE).
  - bf16 output DMA (host accumulates the two partials in fp32).
  Note: S pairs are emitted in 2-chunk row-tiled batches hoping for PE
  array packing (microbenches show 2x for adjacent (0,0)/(64,0) K=64 pairs)
  but in-kernel the pairs run serial regardless -- packing only engages in
  clean uninterrupted tiled streams; kept because it costs nothing.
"""
import sys, os, contextlib
from collections import deque

for _p in ("/opt/trn_rl_repo", "/root/.axon_site/_ro/trn_rl_repo"):
    if os.path.isdir(_p) and _p not in sys.path:
        sys.path.insert(0, _p)

import numpy as np

T, C, NHEAD, HS = 2048, 1024, 16, 64
NCORES = 8
HPC = NHEAD // 2          # heads per core = 8
DPC = HPC * HS            # head dims per core = 512
NCC = C // 128            # contraction chunks = 8
NQT = T // 512            # q tiles = 4
NPAIR = HPC // 2          # head pairs per core = 4
VLAG = 6                  # S->V chunk pipelining lag (chunks)

_CACHE = {}


def _build():
    import concourse.tile as tile
    import concourse.bass as bass
    from concourse import bacc, mybir

    f32 = mybir.dt.float32
    f32r = mybir.dt.float32r
    bf16 = mybir.dt.bfloat16
    FT = mybir.ActivationFunctionType
    from concourse.masks import make_identity

    nc = bacc.Bacc("TRN2", target_bir_lowering=False)
    xt_d = nc.declare_dram_parameter("xt", [C, T], bf16, isOutput=False)
    wq_d = nc.declare_dram_parameter("wq", [C, DPC], bf16, isOutput=False)
    wk_d = nc.declare_dram_parameter("wk", [C, DPC], bf16, isOutput=False)
    wv_d = nc.declare_dram_parameter("wv", [C, DPC], bf16, isOutput=False)
    wp_d = nc.declare_dram_parameter("wp", [DPC, C], bf16, isOutput=False)
    masks_d = nc.declare_dram_parameter("masks_c", [128, 256], bf16, isOutput=False)
    vones_d = nc.declare_dram_parameter("vones_c", [128, NPAIR, 33], bf16, isOutput=False)
    onesr_d = nc.declare_dram_parameter("onesr_c", [128, 128], bf16, isOutput=False)
    o_d = nc.declare_dram_parameter("o", [T, C], bf16, isOutput=True)

    xt_r = xt_d[:].rearrange("(n p) t -> p n t", p=128)   # [128, 8, 2048]
    o_r = o_d[:].rearrange("(n p) c -> n p c", p=128)

    with tile.TileContext(nc) as tc:
      with contextlib.ExitStack() as top:
        top.enter_context(nc.allow_low_precision(reason="bf16 pipeline, fp32 accum"))
        const = top.enter_context(tc.tile_pool(name="const", bufs=1))
        persist = top.enter_context(tc.tile_pool(name="persist", bufs=1))
        xpool = top.enter_context(tc.tile_pool(name="xpool", bufs=3))
        ppool = top.enter_context(tc.tile_pool(name="ppool", bufs=VLAG + 4))
        dpool = top.enter_context(tc.tile_pool(name="dpool", bufs=2))
        dbpool = top.enter_context(tc.tile_pool(name="dbpool", bufs=2))
        opool = top.enter_context(tc.tile_pool(name="opool", bufs=2))
        pss = top.enter_context(tc.tile_pool(name="pss", bufs=2, space="PSUM"))
        psy = top.enter_context(tc.tile_pool(name="psy", bufs=2, space="PSUM"))
        psmm = top.enter_context(tc.tile_pool(name="psmm", bufs=2, space="PSUM"))

        ident = const.tile([128, 128], bf16, tag="ident")
        make_identity(nc, ident)

        # HAM pre-warm during the initial DMA wait
        for _w in range(16):
            pwt = psmm.tile([128, 128], f32, tag="mm")
            nc.tensor.matmul(pwt[:], ident[:], ident[:], start=True, stop=True)

        # persistent tensors
        qT = [persist.tile([128, T], bf16, tag=f"qT{u}", name=f"qT{u}") for u in range(NPAIR)]
        kT = [persist.tile([128, T], bf16, tag=f"kT{u}", name=f"kT{u}") for u in range(NPAIR)]
        vp = [persist.tile([128, NPAIR, 161], bf16, tag=f"vp{t}", name=f"vp{t}")
              for t in range(T // 128)]
        yT = [persist.tile([128, T], bf16, tag=f"yT{u}", name=f"yT{u}") for u in range(NPAIR)]

        wq_sb = persist.tile([128, NCC, DPC], bf16, tag="wq")
        wk_sb = persist.tile([128, NCC, DPC], bf16, tag="wk")
        wv_sb = persist.tile([128, NCC, DPC], bf16, tag="wv")
        wp_sb = persist.tile([128, NPAIR, C], bf16, tag="wp")

        # ones rows for the 64-row-mode denominator broadcast matmuls
        onesr = const.tile([128, 128], bf16, tag="onesr")
        nc.sync.dma_start(out=onesr, in_=onesr_d[:])

        # ---------------- startup DMAs ----------------
        xq = {}   # quad -> [128, 8, 512] xT slice

        def dma_quad_x(q):
            xn = xpool.tile([128, NCC, 512], bf16, tag="xq")
            nc.sync.dma_start(out=xn, in_=xt_r[:, :, 512 * q:512 * (q + 1)])
            xq[q] = xn

        dma_quad_x(0)
        wqr = wq_d[:].rearrange("(n p) d -> p n d", p=128)
        wkr = wk_d[:].rearrange("(n p) d -> p n d", p=128)
        nc.sync.dma_start(out=wq_sb[:, :, 0:128], in_=wqr[:, :, 0:128])
        nc.sync.dma_start(out=wk_sb[:, :, 0:128], in_=wkr[:, :, 0:128])
        nc.sync.dma_start(out=wq_sb[:, :, 128:512], in_=wqr[:, :, 128:512])
        nc.sync.dma_start(out=wk_sb[:, :, 128:512], in_=wkr[:, :, 128:512])
        nc.sync.dma_start(out=wv_sb, in_=wv_d[:].rearrange("(n p) d -> p n d", p=128))
        masks = const.tile([128, 2, 128], bf16, tag="masks")
        nc.sync.dma_start(out=masks, in_=masks_d[:].rearrange("p (h n) -> p h n", h=2))
        for t in range(T // 128):
            nc.sync.dma_start(out=vp[t][:, :, 64:97], in_=vones_d[:])
        nc.sync.dma_start(out=wp_sb, in_=wp_d[:].rearrange("(n p) c -> p n c", p=128))

        # ---------------- qkv quad machinery ----------------
        def quad_closures(q):
            ops = []
            qs = slice(512 * q, 512 * (q + 1))

            def mk_qk(dt, which):
                w_sb, dst = (wq_sb, qT) if which == "q" else (wk_sb, kT)

                def f():
                    ps = psmm.tile([128, 512], f32, tag="mm")
                    for cc in range(NCC):
                        nc.tensor.matmul(ps[:], w_sb[:, cc, dt * 128:(dt + 1) * 128],
                                         xq[q][:, cc, :], start=(cc == 0), stop=(cc == NCC - 1))
                    nc.vector.tensor_copy(dst[dt][:, qs], ps[:])
                return f

            def mk_v(j):
                tt = 4 * q + j

                def f():
                    ps = psmm.tile([128, 512], f32, tag="mm")
                    for cc in range(NCC):
                        nc.tensor.matmul(ps[:], xq[q][:, cc, j * 128:(j + 1) * 128],
                                         wv_sb[:, cc, :], start=(cc == 0), stop=(cc == NCC - 1))
                    pv = ps.rearrange("p (u two d) -> p u two d", u=NPAIR, two=2)
                    nc.vector.tensor_copy(vp[tt][:, :, 0:64], pv[:, :, 0, :])
                    nc.vector.tensor_copy(vp[tt][:, :, 97:161], pv[:, :, 1, :])
                return f

            for dt in range(4):
                ops.append((("qk", q, dt, "q"), mk_qk(dt, "q")))
                ops.append((("qk", q, dt, "k"), mk_qk(dt, "k")))
            for j in range(4):
                ops.append((("v", q, j), mk_v(j)))
            return ops

        # ---------------- attention ----------------
        pending_norm = deque()   # deferred 64-row-mode norm MM pairs + muls

        def emit_pending_norms():
            while pending_norm:
                fn = pending_norm.popleft()
                fn()

        def mk_norm(u, i, dinvb):
            qs = slice(512 * i, 512 * (i + 1))

            def f():
                # packed 64-row-mode pair: replicate 1/den_e (row 64) and
                # 1/den_o (row 32) across all psum partitions
                rbE = psmm.tile([128, 512], f32, tag="mm")
                nc.tensor.matmul(rbE[:], onesr[64:128, :], dinvb[64:128, :],
                                 start=True, stop=True, tile_position=(64, 0))
                rbO = psmm.tile([128, 512], f32, tag="mm")
                nc.tensor.matmul(rbO[:], onesr[0:64, :], dinvb[0:64, :],
                                 start=True, stop=True, tile_position=(0, 0))
                nc.vector.tensor_mul(yT[u][0:64, qs], yT[u][0:64, qs], rbE[0:64, :])
                nc.vector.tensor_mul(yT[u][64:128, qs], yT[u][64:128, qs], rbO[64:128, :])
            return f

        def emit_attn(u, i, pop):
            L = 4 * (i + 1)
            qs = slice(512 * i, 512 * (i + 1))
            ps_e = psy.tile([128, 512], f32, tag="ps_y")
            ps_o = psy.tile([128, 512], f32, tag="ps_y")
            Pt = {}
            Nc = {}   # chunk -> valid q-column start within the 512-wide tile

            def pv(c):
                lo = Nc[c]
                nc.tensor.matmul(ps_e[0:65, lo:512], vp[c][:, u, 0:65],
                                 Pt[c][:, lo:512],
                                 start=(c == 0), stop=(c == L - 1))
                nc.tensor.matmul(ps_o[:, lo:512], vp[c][:, u, 33:161],
                                 Pt[c][:, 512 + lo:1024],
                                 start=(c == 0), stop=(c == L - 1))

            for g in range(L // 2):
                c0, c1 = 2 * g, 2 * g + 1
                # ---- 64-row-mode batch: pending norms + two S pairs ----
                # (flush at g==1 so the previous unit's reciprocal has had a
                # full group of slack before the PE hits the rb matmuls)
                if g == 1:
                    emit_pending_norms()
                sps = {}
                for c in (c0, c1):
                    j = c - 4 * i
                    lo = 128 * j if j > 0 else 0
                    Nc[c] = lo
                    sp = pss.tile([128, 1024], f32, tag="s_pair")
                    nc.tensor.matmul(sp[:, lo:512], kT[u][0:64, c * 128:(c + 1) * 128],
                                     qT[u][0:64, 512 * i + lo:512 * (i + 1)],
                                     start=True, stop=True, tile_position=(0, 0))
                    nc.tensor.matmul(sp[:, 512 + lo:1024], kT[u][64:128, c * 128:(c + 1) * 128],
                                     qT[u][64:128, 512 * i + lo:512 * (i + 1)],
                                     start=True, stop=True, tile_position=(64, 0))
                    sps[c] = sp
                # ---- exp + masks ----
                for c in (c0, c1):
                    j = c - 4 * i
                    lo = Nc[c]
                    P = ppool.tile([128, 1024], bf16, tag="P")
                    if lo == 0:
                        nc.scalar.activation(out=P[:], in_=sps[c][:], func=FT.Exp,
                                             scale=float(HS) ** -0.5)
                    else:
                        nc.scalar.activation(out=P[:, lo:512], in_=sps[c][:, lo:512],
                                             func=FT.Exp, scale=float(HS) ** -0.5)
                        nc.scalar.activation(out=P[:, 512 + lo:1024],
                                             in_=sps[c][:, 512 + lo:1024],
                                             func=FT.Exp, scale=float(HS) ** -0.5)
                    if j >= 0:
                        # causal mask on the 128-wide diagonal block via
                        # gpsimd (idle engine; keeps PV's dep off the DVE
                        # queue): keep col >= row, else 0
                        p3 = P.rearrange("p (h n) -> p h n", h=2)
                        nc.gpsimd.affine_select(
                            out=p3[:, :, lo:lo + 128], in_=p3[:, :, lo:lo + 128],
                            pattern=[[0, 2], [1, 128]],
                            compare_op=mybir.AluOpType.is_ge, fill=0.0,
                            base=0, channel_multiplier=-1)
                    Pt[c] = P
                # ---- lagged PV (full-array mode) ----
                if c1 - VLAG >= 0:
                    pv(c0 - VLAG)
                    pv(c1 - VLAG)
                pop()
            for c in range(max(0, L - VLAG), L):
                pv(c)
            # ---- epilogue: recips via staged SBUF den rows, unnormalized yT ----
            # (single full-tile recip at base partition 0: custom DVE ops at
            # a nonzero base partition mis-lower and corrupt SBUF)
            dtmp = dpool.tile([128, 512], f32, tag="dtmp")
            drec = dpool.tile([128, 512], f32, tag="drec")
            nc.gpsimd.memset(dtmp[:], 1.0)
            nc.vector.tensor_copy(dtmp[64:65, :], ps_e[64:65, :])
            nc.vector.tensor_copy(dtmp[32:33, :], ps_o[32:33, :])
            nc.vector.reciprocal_approx_fast(out=drec[:], in_=dtmp[:])
            dinvb = dbpool.tile([128, 512], bf16, tag="dinvb")
            nc.gpsimd.memset(dinvb[:], 1.0)
            nc.vector.tensor_copy(dinvb[32:33, :], drec[32:33, :])
            nc.vector.tensor_copy(dinvb[64:65, :], drec[64:65, :])
            nc.vector.tensor_copy(yT[u][0:64, qs], ps_e[0:64, :])
            nc.vector.tensor_copy(yT[u][64:128, qs], ps_o[64:128, :])
            pending_norm.append(mk_norm(u, i, dinvb))

        # ---------------- projection ----------------
        def proj_closures(i):
            ops = []

            def mk_proj(tt):
                def f():
                    out_sb = opool.tile([128, C], bf16, tag="out_sb")
                    for ct in range(2):
                        po = psmm.tile([128, 512], f32, tag="mm")
                        for u in range(NPAIR):
                            nc.tensor.matmul(po[:], yT[u][:, tt * 128:(tt + 1) * 128],
                                             wp_sb[:, u, ct * 512:(ct + 1) * 512],
                                             start=(u == 0), stop=(u == NPAIR - 1))
                        nc.vector.tensor_copy(out_sb[:, ct * 512:(ct + 1) * 512], po[:])
                    nc.sync.dma_start(out=o_r[tt], in_=out_sb)
                return f

            for tt in range(4 * i, 4 * i + 4):
                ops.append((("proj", i, tt), mk_proj(tt)))
            return ops

        # ---------------- emission schedule ----------------
        q0 = quad_closures(0)
        prologue_keys = {("qk", 0, 0, "q"), ("qk", 0, 0, "k")}
        prologue_keys |= {("v", 0, j) for j in range(4)}
        rest0 = []
        for key, cl in q0:
            if key in prologue_keys:
                cl()
            else:
                rest0.append((key, cl))

        fillers = deque(rest0)

        def make_pop(total_groups, spill=0):
            state = {"left": total_groups + spill}

            def pop():
                n = state["left"]
                if n > 0:
                    k = len(fillers) // n
                    for _ in range(min(k, len(fillers))):
                        key, cl = fillers.popleft()
                        cl()
                    state["left"] = n - 1
            return pop

        dma_quad_x(1)
        for i in range(NQT):
            if i < NQT - 2:
                dma_quad_x(i + 2)
            if i < NQT - 1:
                fillers.extend(quad_closures(i + 1))
            if i > 0:
                # stage i-1's yT fully normalized once its last unit's norm
                # flushes; then its proj closures can pop as fillers
                emit_pending_norms()
                fillers.extend(proj_closures(i - 1))
            pop = make_pop(NPAIR * 2 * (i + 1), spill=6)
            for u in range(NPAIR):
                need = {("qk", qq, u, w) for qq in range(i + 1) for w in ("q", "k")}
                need |= {("v", qq, j) for qq in range(i + 1) for j in range(4)}
                while any(key in need for key, _ in fillers):
                    key, cl = fillers.popleft()
                    cl()
                emit_attn(u, i, pop)
        # tail: drain leftovers first (covers the last norm's recip latency),
        # then the pending norm, then the final projection
        while fillers:
            key, cl = fillers.popleft()
            cl()
        emit_pending_norms()
        for key, cl in proj_closures(NQT - 1):
            cl()

    nc.compile()
    return nc


def _get_nc():
    if "nc" not in _CACHE:
        _CACHE["nc"] = _build()
    return _CACHE["nc"]


def _in_maps(x, W_attn, W_proj):
    import ml_dtypes
    bf16 = ml_dtypes.bfloat16
    a_idx = np.arange(128)[:, None]
    b_idx = np.arange(128)[None, :]
    m = (b_idx >= a_idx).astype(np.float32)                 # [128k, 128q] valid q>=k
    masks_c = np.ascontiguousarray(
        np.concatenate([m, m], axis=1).astype(bf16))        # [128, 256]
    vones_c = np.ones((128, NPAIR, 33), bf16)
    onesr_c = np.zeros((128, 128), np.float32)
    onesr_c[32, :] = 1.0
    onesr_c[64, :] = 1.0
    onesr_c = np.ascontiguousarray(onesr_c.astype(bf16))
    maps = []
    for core in range(NCORES):
        b, g = core // 2, core % 2
        cs = slice(DPC * g, DPC * (g + 1))
        maps.append({
            "xt": np.ascontiguousarray(x[b].T.astype(bf16)),
            "wq": np.ascontiguousarray(W_attn[:, cs].astype(bf16)),
            "wk": np.ascontiguousarray(W_attn[:, C:][:, cs].astype(bf16)),
            "wv": np.ascontiguousarray(W_attn[:, 2 * C:][:, cs].astype(bf16)),
            "wp": np.ascontiguousarray(W_proj[cs, :].astype(bf16)),
            "masks_c": masks_c,
            "vones_c": vones_c,
            "onesr_c": onesr_c,
        })
    return maps


def _install_ntff_shim():
    """Provide antenv.axon_hooks (absent in this image) so trace=True works."""
    import sys as _sys, types, ctypes, contextlib as _cl
    if "antenv.axon_hooks" in _sys.modules:
        return
    so_path = "/opt/axon/libaxon_pjrt.so"
    try:
        lib = ctypes.CDLL(so_path)
        lib.axon_start_nrt_profile.argtypes = [ctypes.POINTER(ctypes.c_int64), ctypes.c_size_t]
        lib.axon_start_nrt_profile.restype = ctypes.c_int64
        lib.axon_stop_nrt_profile.argtypes = [ctypes.c_char_p]
        lib.axon_stop_nrt_profile.restype = ctypes.c_int64
    except (OSError, AttributeError):
        return

    @_cl.contextmanager
    def _hook(output_dir, device_ids):
        import jax
        jax.devices()
        if device_ids:
            ids = (ctypes.c_int64 * len(device_ids))(*device_ids)
            rc = lib.axon_start_nrt_profile(ids, len(device_ids))
        else:
            rc = lib.axon_start_nrt_profile(None, 0)
        if rc != 0:
            raise RuntimeError(f"axon_start_nrt_profile rc={rc}")
        try:
            yield
        finally:
            n = lib.axon_stop_nrt_profile(str(output_dir).encode())
            if n < 0:
                raise RuntimeError(f"axon_stop_nrt_profile rc={n}")

    mod = types.ModuleType("antenv.axon_hooks")
    mod.get_axon_ntff_profile_hook = lambda: _hook
    mod.set_axon_ntff_profile_hook = lambda h: None
    _sys.modules["antenv.axon_hooks"] = mod


def kernel(x, W_attn, W_proj, _trace=False):
    from concourse.bass_utils import run_bass_kernel_spmd
    if _trace:
        _install_ntff_shim()
    x = np.asarray(x, dtype=np.float32)
    W_attn = np.asarray(W_attn, dtype=np.float32)
    W_proj = np.asarray(W_proj, dtype=np.float32)
    nc = _get_nc()
    res = run_bass_kernel_spmd(nc, _in_maps(x, W_attn, W_proj),
                               core_ids=list(range(NCORES)), trace=_trace)
    out = np.empty((4, T, C), np.float32)
    for b in range(4):
        out[b] = (res.results[2 * b]["o"].astype(np.float32)
                  + res.results[2 * b + 1]["o"].astype(np.float32))
    if _trace:
        return out, res
    return out
